# revision 1
# baseline (speedup 1.0000x reference)
"""Trainium2 Bass kernel for fused attention block (B=2, S=2048, H=1024, N=16, D=64).

Sharding: 8 cores = 2 batches (DP) x 4 head-groups (TP, 4 heads each).
Per core: q/kv projections + LN + RoPE + attention for its 4 heads, AllGather
of normalized attention outputs (bf16) within the batch quad (split in two so
the first gather overlaps attention), then a 256-column slice of the output
projection.

Pipeline: Q path runs first (proj -> LN/rope -> DMA-transpose), then the KV
projection streams on PE while attention (ACT-bound exp) consumes per-head
K tiles as they become ready. PV keeps V' (with an extra ones column for the
softmax sums) stationary so probs tiles die immediately after each t-block.

QK stationaries are zero-padded to K=128 (full array) so the PE activity
monitor sees full-width matmuls; biases ride the projection matmul as a K=1
ones-row term; psum evacuation runs on ScalarE to keep the DVE free for the
LN/rope chain.
"""

import numpy as np
import ml_dtypes

import concourse.bass as bass
from concourse import bacc
import concourse.mybir as mybir
import concourse.tile as tile
from concourse.masks import make_identity

# problem shape (hardcoded per contract)
B, S, H, NH, D = 2, 2048, 1024, 16, 64
EPS = 1.0 / 65530.0
NCORES = 8
HPC = 4            # heads per core
OC = HPC * D       # 256 head-dims per core
P = 128
SB = S // P        # 16 s-blocks
KC = H // P        # 8 contraction chunks of 128
D2 = D // 2
SCALE = 1.0 / 8.0  # 1/sqrt(D)
DV = D + 1         # V columns per head incl. ones column
SC = 512           # s-chunk for PV accumulation
NSC = S // SC      # 4
HTC = 4            # hT DMA chunks along s

BF = mybir.dt.bfloat16
F32 = mybir.dt.float32
I32 = mybir.dt.int32
ALU = mybir.AluOpType
ACTF = mybir.ActivationFunctionType

# Schraudolph fast-exp constants (int32 bit trick), calibrated for min max
# relative error (~3%); used to offload part of the softmax exp to the DVE
EXP_A = float(2**23 / np.log(2))
EXP_B = float(127 * 2**23 - 366400)
DVE_EXP_T = (1, 4, 7, 10, 13)   # t-blocks whose exp runs on the DVE
MUC = 8                          # extra proj columns carrying per-head means
PW = 3 * OC + MUC                # projection psum width


def build_nc():
    nc = bacc.Bacc(num_devices=NCORES)

    hT = nc.declare_dram_parameter("hT", [H, S], BF, isOutput=False)
    qwT = nc.declare_dram_parameter("qwT", [H, OC], BF, isOutput=False)
    kwT = nc.declare_dram_parameter("kwT", [H, OC], BF, isOutput=False)
    vwT = nc.declare_dram_parameter("vwT", [H, OC], BF, isOutput=False)
    owT = nc.declare_dram_parameter("owT", [H, OC], BF, isOutput=False)
    muwT = nc.declare_dram_parameter("muwT", [H, MUC], BF, isOutput=False)
    qkvb = nc.declare_dram_parameter("qkvb", [1, PW], BF, isOutput=False)
    ob = nc.declare_dram_parameter("ob", [P, OC], F32, isOutput=False)
    cosd = nc.declare_dram_parameter("cosd", [S, D], F32, isOutput=False)
    sind = nc.declare_dram_parameter("sind", [S, D], F32, isOutput=False)
    out = nc.declare_dram_parameter("out", [S, OC], F32, isOutput=True)

    with tile.TileContext(nc) as tc:
        with tc.tile_pool(name="persist", bufs=1) as persist:
            # warm-up fodder + identity first so PE can start immediately
            junk = persist.tile([P, 512], BF)
            nc.gpsimd.memset(junk[:], 1.0)
            ident = persist.tile([P, P], BF)
            make_identity(nc, ident)
            onesrow = persist.tile([1, P], BF)
            nc.gpsimd.memset(onesrow[:], 1.0)
            onescol64 = persist.tile([P, D], BF)
            nc.gpsimd.memset(onescol64[64:65, :], 1.0)

            cos_sb = persist.tile([P, SB, D], F32)
            nc.scalar.dma_start(cos_sb[:], cosd[:].rearrange("(a p) d -> p a d", p=P))
            sin_sb = persist.tile([P, SB, D], F32)
            nc.scalar.dma_start(sin_sb[:], sind[:].rearrange("(a p) d -> p a d", p=P))
            qkvb_sb = persist.tile([1, PW], BF)
            nc.scalar.dma_start(qkvb_sb[:], qkvb[:])
            ob_sb = persist.tile([P, OC], F32)
            nc.scalar.dma_start(ob_sb[:], ob[:])
            # cos/sin pre-broadcast over heads in bf16 so the whole rope
            # chain runs contiguous-bf16 (2x DVE mode)
            cos_f = persist.tile([P, SB, HPC, D], BF)
            nc.vector.tensor_copy(
                out=cos_f[:], in_=cos_sb[:, :, None, :].to_broadcast((P, SB, HPC, D))
            )
            sin_f = persist.tile([P, SB, HPC, D], BF)
            nc.vector.tensor_copy(
                out=sin_f[:], in_=sin_sb[:, :, None, :].to_broadcast((P, SB, HPC, D))
            )

            # transposed q, one full-K chunk per head: head h occupies rows
            # (h%2)*64..+64 of chunk h, the other 64 rows stay zero so the QK
            # matmul can use the full-K k-pair chunk as stationary (the zero
            # q rows cancel the other head's contribution)
            qT2z = persist.tile([P, HPC, S], BF)
            nc.gpsimd.memset(qT2z[0:64, 1::2], 0.0)
            nc.gpsimd.memset(qT2z[64:P, 0::2], 0.0)
            # transposed k in head-pair chunks: chunk c rows 0..63 = head 2c,
            # rows 64..127 = head 2c+1
            kT2 = persist.tile([P, 2, S], BF)
            # v in [s, head*(D+1)] layout: D data cols + 1 ones col per head
            Vp = persist.tile([P, SB, HPC * DV], BF)
            for h in range(HPC):
                nc.gpsimd.memset(Vp[:, :, h * DV + D : (h + 1) * DV], 1.0)
            attnT = persist.tile([D, HPC, S], BF)      # normalized [d, h, s]
            owT_sb = persist.tile([P, KC, OC], BF)

            mu_q = persist.tile([P, SB, HPC], F32)
            mu_k = persist.tile([P, SB, HPC], F32)
            var_q = persist.tile([P, SB, HPC], F32)
            var_k = persist.tile([P, SB, HPC], F32)
            rstd_q = persist.tile([P, SB, HPC], F32)
            rstd_k = persist.tile([P, SB, HPC], F32)
            std_q = persist.tile([P, SB, HPC], F32)
            std_k = persist.tile([P, SB, HPC], F32)
            eps_t = persist.tile([P, 1], F32)
            nc.gpsimd.memset(eps_t[:], EPS)

            GS = 4                    # s-blocks per prologue pipeline group
            NG = SB // GS

            def stats_grp(xf, g, mu, var, pool, tagp):
                gs = slice(g * GS, (g + 1) * GS)
                sqf = pool.tile([P, GS, OC], F32, name=f"sqf{tagp}{g}", tag="sqf", bufs=2)
                nc.scalar.activation(sqf[:], xf[:, gs], ACTF.Square)
                sv = sqf[:].rearrange("p s (h d) -> p s h d", h=HPC)
                nc.vector.tensor_reduce(out=var[:, gs], in_=sv, axis=mybir.AxisListType.X, op=ALU.add)
                nc.vector.tensor_scalar_mul(var[:, gs], var[:, gs], 1.0 / D)
                mu2 = pool.tile([P, GS, HPC], F32, name=f"mu2{tagp}{g}", tag="mu2", bufs=2)
                nc.vector.tensor_tensor(out=mu2[:], in0=mu[:, gs], in1=mu[:, gs], op=ALU.mult)
                nc.vector.tensor_tensor(out=var[:, gs], in0=var[:, gs], in1=mu2[:], op=ALU.subtract)

            def ln_rope_transpose_grp(xf, g, mu, rstd, pool, tagp, pe_transpose=None):
                gs = slice(g * GS, (g + 1) * GS)
                xv = xf[:, gs].rearrange("p s (h d) -> p s h d", h=HPC)
                # expand per-(s,h) stats to full contiguous bf16 tiles so the
                # apply chain runs in 2x DVE mode
                mu_f = pool.tile([P, GS, HPC, D], BF, name=f"muf{tagp}{g}", tag="muf", bufs=2)
                nc.vector.tensor_copy(
                    out=mu_f[:], in_=mu[:, gs, :, None].to_broadcast((P, GS, HPC, D))
                )
                rs_f = pool.tile([P, GS, HPC, D], BF, name=f"rsf{tagp}{g}", tag="rsf", bufs=2)
                nc.vector.tensor_copy(
                    out=rs_f[:], in_=rstd[:, gs, :, None].to_broadcast((P, GS, HPC, D))
                )
                nc.vector.tensor_tensor(out=xv, in0=xv, in1=mu_f[:], op=ALU.subtract)
                nc.vector.tensor_tensor(out=xv, in0=xv, in1=rs_f[:], op=ALU.mult)
                cb = cos_f[:, gs]
                s1 = sin_f[:, gs, :, 0:D2]
                s2 = sin_f[:, gs, :, D2:D]
                ca = pool.tile([P, GS, HPC, D], BF, name=f"ca{tagp}{g}", tag="ca", bufs=2)
                th = pool.tile([P, GS, HPC, D2], BF, name=f"th{tagp}{g}", tag="th", bufs=2)
                t2 = pool.tile([P, GS, HPC, D2], BF, name=f"t2{tagp}{g}", tag="t2", bufs=2)
                rx = pool.tile([P, GS, HPC, D], BF, name=f"rx{tagp}{g}", tag="rx", bufs=2)
                nc.gpsimd.tensor_tensor(out=th[:], in0=xv[:, :, :, D2:D], in1=s1, op=ALU.mult)
                nc.gpsimd.tensor_tensor(out=t2[:], in0=xv[:, :, :, 0:D2], in1=s2, op=ALU.mult)
                nc.gpsimd.tensor_tensor(out=ca[:], in0=xv, in1=cb, op=ALU.mult)
                nc.vector.tensor_tensor(out=rx[:, :, :, 0:D2], in0=ca[:, :, :, 0:D2], in1=th[:], op=ALU.subtract)
                nc.vector.tensor_tensor(out=rx[:, :, :, D2:D], in0=ca[:, :, :, D2:D], in1=t2[:], op=ALU.add)
                rx2 = rx[:].rearrange("p s h d -> p s (h d)")
                for c in range(2):
                    for si in range(GS):
                        sb = g * GS + si
                        if pe_transpose is not None:
                            tpool, ident = pe_transpose
                            pst = tpool.tile([P, P], BF, name=f"pst{tagp}{c}{sb}", tag="pst")
                            nc.tensor.transpose(pst[:], rx2[:, si, c * P : (c + 1) * P], ident[:])
                            # split the pair transpose into the zero-padded
                            # full-K per-head chunks
                            nc.vector.tensor_copy(
                                out=qT2z[0:64, 2 * c, sb * P : (sb + 1) * P],
                                in_=pst[0:64, :],
                            )
                            nc.vector.tensor_copy(
                                out=qT2z[64:P, 2 * c + 1, sb * P : (sb + 1) * P],
                                in_=pst[64:P, :],
                            )
                        else:
                            nc.sync.dma_start(
                                kT2[:, c, sb * P : (sb + 1) * P],
                                rx2[:, si, c * P : (c + 1) * P],
                                transpose=True,
                            )

            # ---------------- phase Q ----------------------------------
            with tc.tile_pool(name="pw", bufs=1) as pw, \
                 tc.tile_pool(name="projpsum", bufs=3, space="PSUM") as projpsum, \
                 tc.tile_pool(name="tpsum", bufs=2, space="PSUM") as tpsum, \
                 tc.tile_pool(name="ptmp", bufs=3) as ptmp:
                # weights first (needed by the first matmul), then hT in
                # s-chunks so the first projection group starts early
                qkvwT_sb = pw.tile([P, KC, PW], BF)
                nc.sync.dma_start(qkvwT_sb[:, :, 0:OC], qwT[:].rearrange("(a p) o -> p a o", p=P))
                nc.sync.dma_start(qkvwT_sb[:, :, OC : 2 * OC], kwT[:].rearrange("(a p) o -> p a o", p=P))
                nc.sync.dma_start(qkvwT_sb[:, :, 2 * OC : 3 * OC], vwT[:].rearrange("(a p) o -> p a o", p=P))
                nc.sync.dma_start(qkvwT_sb[:, :, 3 * OC : PW], muwT[:].rearrange("(a p) o -> p a o", p=P))
                hT_sb = pw.tile([P, KC, S], BF)
                SCH = S // HTC
                for hc in range(HTC):
                    nc.sync.dma_start(
                        hT_sb[:, :, hc * SCH : (hc + 1) * SCH],
                        hT[:, hc * SCH : (hc + 1) * SCH].rearrange("(a p) s -> p a s", p=P),
                    )
                nc.sync.dma_start(owT_sb[:], owT[:].rearrange("(a p) o -> p a o", p=P))

                # PE warm-up: sustained matmul burst releases the HAM clock
                # throttle and bridges the initial DMA wait
                wps = projpsum.tile([P, PW], F32, name="wps", tag="pq")
                for _ in range(12):
                    nc.tensor.matmul(wps[:, 0:512], ident[:], junk[:], start=True, stop=True)

                qf = pw.tile([P, SB, OC], BF)
                kf = pw.tile([P, SB, OC], BF)

                def q_chain(g):
                    gsl = slice(g * GS, (g + 1) * GS)
                    stats_grp(qf, g, mu_q, var_q, ptmp, "q")
                    nc.scalar.activation(std_q[:, gsl], var_q[:, gsl], ACTF.Sqrt, bias=eps_t[:])
                    nc.vector.reciprocal(rstd_q[:, gsl], std_q[:, gsl])
                    nc.vector.tensor_scalar_mul(rstd_q[:, gsl], rstd_q[:, gsl], SCALE)
                    ln_rope_transpose_grp(qf, g, mu_q, rstd_q, ptmp, "q",
                                          pe_transpose=(tpsum, ident))

                def k_chain(g):
                    gsl = slice(g * GS, (g + 1) * GS)
                    stats_grp(kf, g, mu_k, var_k, ptmp, "k")
                    nc.scalar.activation(std_k[:, gsl], var_k[:, gsl], ACTF.Sqrt, bias=eps_t[:])
                    nc.vector.reciprocal(rstd_k[:, gsl], std_k[:, gsl])
                    ln_rope_transpose_grp(kf, g, mu_k, rstd_k, ptmp, "k")


                for g in range(NG):
                    for si in range(GS):
                        sb = g * GS + si
                        pq = projpsum.tile([P, PW], F32, name=f"pq{sb}", tag="pq")
                        for kc in range(KC):
                            lhsp = hT_sb[:, kc, sb * P : (sb + 1) * P]
                            nc.tensor.matmul(
                                pq[:, 0:512], lhsp, qkvwT_sb[:, kc, 0:512],
                                start=(kc == 0), stop=False,
                            )
                            nc.tensor.matmul(
                                pq[:, 512 : PW], lhsp, qkvwT_sb[:, kc, 512 : PW],
                                start=(kc == 0), stop=False,
                            )
                        # bias as a K=1 ones-row term closing the group
                        nc.tensor.matmul(
                            pq[:, 0:512], onesrow[:], qkvb_sb[:, 0:512],
                            start=False, stop=True,
                        )
                        nc.tensor.matmul(
                            pq[:, 512 : PW], onesrow[:], qkvb_sb[:, 512 : PW],
                            start=False, stop=True,
                        )
                        nc.scalar.copy(out=qf[:, sb], in_=pq[:, 0:OC])
                        nc.scalar.copy(out=kf[:, sb], in_=pq[:, OC : 2 * OC])
                        nc.scalar.copy(
                            out=Vp[:, sb].rearrange("p (h e) -> p h e", h=HPC)[:, :, 0:D],
                            in_=pq[:, 2 * OC : 3 * OC].rearrange("p (h d) -> p h d", h=HPC),
                        )
                        nc.scalar.copy(out=mu_q[:, sb], in_=pq[:, 3 * OC : 3 * OC + HPC])
                        nc.scalar.copy(out=mu_k[:, sb], in_=pq[:, 3 * OC + HPC : PW])
                    if g == 0:
                        k_chain(0)
                    elif g == 1:
                        k_chain(1)
                    elif g == 2:
                        q_chain(0)
                # chain order: k0/k1 first (they gate the first attention
                # t-blocks), then all q (the q-transposes gate the attention
                # start), then k2/k3 — interleaved into the projection loop
                # above via emit_chain so they hide under the proj matmuls
                for fn in (lambda: q_chain(1), lambda: q_chain(2),
                           lambda: q_chain(3), lambda: k_chain(2),
                           lambda: k_chain(3)):
                    fn()

            # ---------------- phase A: attention ------------------------
            with tc.tile_pool(name="dram", bufs=1, space="DRAM") as dram:
                cc_in0 = dram.tile([P, S], BF, name="ccin0")
                cc_out0 = dram.tile([4 * P, S], BF, name="ccout0")
                cc_inh = [dram.tile([D, S], BF, name=f"ccinh{j}") for j in range(2)]
                cc_outh = [dram.tile([4 * D, S], BF, name=f"ccouth{j}") for j in range(2)]

                with tc.tile_pool(name="probs", bufs=8) as probspool, \
                     tc.tile_pool(name="spsum", bufs=2, space="PSUM") as spsum, \
                     tc.tile_pool(name="pvpsum", bufs=1, space="PSUM") as pvpsum, \
                     tc.tile_pool(name="atmp", bufs=6) as atmp:

                    def qk_exp(h, t, probs_t):
                        lhs = kT2[:, h // 2, t * P : (t + 1) * P]
                        for half in range(2):
                            ssc = spsum.tile([P, S // 2], F32, name=f"ssc{h}{t}{half}", tag="ssc")
                            for q4 in range(2):
                                o0 = half * 1024 + q4 * 512
                                nc.tensor.matmul(
                                    ssc[:, q4 * 512 : (q4 + 1) * 512],
                                    lhs,
                                    qT2z[:, h, o0 : o0 + 512],
                                    start=True, stop=True,
                                )
                            if t in DVE_EXP_T:
                                # Schraudolph fast exp on the DVE to offload
                                # the ScalarE (the attention-phase pacer)
                                it = atmp.tile([P, S // 2], I32, name=f"it{h}{t}{half}", tag="it", bufs=2)
                                nc.vector.tensor_scalar(
                                    out=it[:], in0=ssc[:], scalar1=EXP_A, scalar2=EXP_B,
                                    op0=ALU.mult, op1=ALU.add,
                                )
                                nc.vector.tensor_copy(
                                    out=probs_t[:, half * 1024 : (half + 1) * 1024],
                                    in_=it[:].bitcast(F32),
                                )
                            else:
                                nc.scalar.activation(
                                    probs_t[:, half * 1024 : (half + 1) * 1024],
                                    ssc[:], ACTF.Exp,
                                )

                    def pv(h, t, pvp, probs_t):
                        for sc in range(NSC):
                            nc.tensor.matmul(
                                pvp[:, sc * SC : (sc + 1) * SC],
                                Vp[:, t, h * DV : (h + 1) * DV],
                                probs_t[:, sc * SC : (sc + 1) * SC],
                                start=(t == 0), stop=(t == SB - 1),
                            )

                    def normalize_start(h, pvp):
                        # evacuate psum fast (split across DVE+ACT so pvp
                        # frees quickly); reciprocal of the sums row, bf16
                        pvf = atmp.tile([DV, S], F32, name=f"pvf{h}", tag="pvf", bufs=2)
                        nc.vector.tensor_copy(out=pvf[:, 0 : S // 2], in_=pvp[:, 0 : S // 2])
                        nc.scalar.copy(out=pvf[:, S // 2 : S], in_=pvp[:, S // 2 : S])
                        rb = atmp.tile([D, S], F32, name=f"rb{h}", tag="rb", bufs=2)
                        # two-stage tree broadcast: the single-partition sums
                        # row is partition-bandwidth-bound, so fan out 1->8
                        # then 8->56 (reads spread over 8 partitions)
                        nc.sync.dma_start(rb[0:8, :], pvf[D : D + 1, None, :].to_broadcast((1, 8, S)))
                        for i in range(7):
                            nc.sync.dma_start(rb[8 * (i + 1) : 8 * (i + 2), :], rb[0:8, :])
                        return pvf, rb

                    def normalize_finish(h, pvf, rb):
                        nc.vector.reciprocal_approx_fast(rb[:], rb[:])
                        nc.vector.tensor_tensor(
                            out=attnT[:, h, :], in0=pvf[0:D, :], in1=rb[:], op=ALU.mult,
                        )
                        if h == 1:
                            ship_pair0()
                        elif h >= 2:
                            ship_head(h - 2)

                    def ship_pair0():
                        nc.gpsimd.dma_start(
                            cc_in0[:].rearrange("(hh p) s -> p hh s", p=D),
                            attnT[:, 0:2, :],
                        )
                        nc.gpsimd.collective_compute(
                            "AllGather", ALU.bypass,
                            replica_groups=[[0, 1, 2, 3], [4, 5, 6, 7]],
                            ins=[cc_in0[:].opt()], outs=[cc_out0[:].opt()],
                        )

                    def ship_head(j):
                        # heads 2/3 ship individually so the final gather only
                        # waits on the last head's 0.25MB
                        nc.gpsimd.dma_start(cc_inh[j][:], attnT[:, 2 + j, :])
                        nc.gpsimd.collective_compute(
                            "AllGather", ALU.bypass,
                            replica_groups=[[0, 1, 2, 3], [4, 5, 6, 7]],
                            ins=[cc_inh[j][:].opt()], outs=[cc_outh[j][:].opt()],
                        )

                    LAG = 2
                    pending = None
                    for h in range(HPC):
                        pvp = pvpsum.tile([DV, S], F32, name=f"pvp{h}", tag="pvp")
                        probs = {}
                        for t in range(SB):
                            probs[t] = probspool.tile([P, S], BF, name=f"probs_{h}_{t}", tag="probs")
                            qk_exp(h, t, probs[t])
                            if t == 3 and pending is not None:
                                # previous head's normalization finish runs
                                # behind this head's first exps so its DMA
                                # latency never blocks the DVE FIFO
                                normalize_finish(*pending)
                                pending = None
                            if t >= LAG:
                                pv(h, t - LAG, pvp, probs.pop(t - LAG))
                        for t in range(SB - LAG, SB):
                            pv(h, t, pvp, probs.pop(t))
                        pvf, rb = normalize_start(h, pvp)
                        pending = (h, pvf, rb)
                    normalize_finish(*pending)

                # ---------------- phase O: output projection ------------
                # cc_out[i] rows: quad rank g's head pair i -> global o-chunk 2g+i
                with tc.tile_pool(name="opool", bufs=1) as opool, \
                     tc.tile_pool(name="opsum", bufs=8, space="PSUM") as opsum, \
                     tc.tile_pool(name="otmp", bufs=3) as otmp:
                    aT = opool.tile([P, 2, 4, S], BF)   # [p, pair, quadrank, s]
                    nc.scalar.dma_start(aT[:, 0], cc_out0[:].rearrange("(g p) s -> p g s", p=P))
                    nc.scalar.dma_start(aT[0:D, 1], cc_outh[0][:].rearrange("(g p) s -> p g s", p=D))
                    nc.scalar.dma_start(aT[D:P, 1], cc_outh[1][:].rearrange("(g p) s -> p g s", p=D))
                    # accumulate chunks in AllGather-arrival order: pair-0
                    # (K=128) and head-2 rows (K=64) first, head-3 rows last,
                    # with all 16 psum tiles live so only the h3 tail waits on
                    # the final gather
                    psos = []
                    for j in range(SB // 2):
                        pso = opsum.tile([P, 2, OC], F32, name=f"pso{j}", tag="pso")
                        psos.append(pso)
                        for half in range(2):
                            sb = 2 * j + half
                            for g in range(4):
                                # start=True only on the bank's very first
                                # matmul: it clears has_written for the WHOLE
                                # bank, so the second half must not re-clear
                                nc.tensor.matmul(
                                    pso[:, half],
                                    aT[:, 0, g, sb * P : (sb + 1) * P],
                                    owT_sb[:, 2 * g],
                                    start=(g == 0 and half == 0), stop=False,
                                )
                            for g in range(4):
                                nc.tensor.matmul(
                                    pso[:, half],
                                    aT[0:D, 1, g, sb * P : (sb + 1) * P],
                                    owT_sb[0:D, 2 * g + 1],
                                    start=False, stop=False,
                                )
                    for sb in range(SB):
                        for g in range(4):
                            nc.tensor.matmul(
                                psos[sb // 2][:, sb % 2],
                                aT[D:P, 1, g, sb * P : (sb + 1) * P],
                                owT_sb[D:P, 2 * g + 1],
                                start=False, stop=(g == 3),
                            )
                        of = otmp.tile([P, OC], F32, name=f"of{sb}", tag="of")
                        nc.vector.tensor_tensor(out=of[:], in0=psos[sb // 2][:, sb % 2], in1=ob_sb[:], op=ALU.add)
                        nc.scalar.dma_start(out[sb * P : (sb + 1) * P, :], of[:])

    nc.finalize()
    return nc


_NC_CACHE = None


def _get_nc():
    global _NC_CACHE
    if _NC_CACHE is None:
        _NC_CACHE = build_nc()
    return _NC_CACHE


def _prep_in_maps(inputs):
    bf16 = ml_dtypes.bfloat16
    hidden = np.asarray(inputs["hidden_states"], np.float32)
    cos = np.ascontiguousarray(np.asarray(inputs["cos"], np.float32))
    sin = np.ascontiguousarray(np.asarray(inputs["sin"], np.float32))
    q_w = np.asarray(inputs["q_w"], np.float32)
    q_b = np.asarray(inputs["q_b"], np.float32)
    kv_w = np.asarray(inputs["kv_w"], np.float32)
    kv_b = np.asarray(inputs["kv_b"], np.float32)
    o_w = np.asarray(inputs["o_w"], np.float32)
    o_b = np.asarray(inputs["o_b"], np.float32)

    hT = [np.ascontiguousarray(hidden[b].T).astype(bf16) for b in range(B)]

    in_maps = []
    for c in range(NCORES):
        b, hg = divmod(c, 4)
        sl = slice(hg * OC, (hg + 1) * OC)
        vsl = slice(H + hg * OC, H + (hg + 1) * OC)
        # per-head averaged weight rows: the projection matmul then emits the
        # LN means directly (columns 3*OC..3*OC+7 of the psum)
        qw_h = q_w[sl].reshape(HPC, D, H).mean(axis=1)        # [4, H]
        kw_h = kv_w[sl].reshape(HPC, D, H).mean(axis=1)       # [4, H]
        muw = np.concatenate([qw_h, kw_h], axis=0)            # [8, H]
        qb_mu = q_b[sl].reshape(HPC, D).mean(axis=1)
        kb_mu = kv_b[sl].reshape(HPC, D).mean(axis=1)
        qkvb_row = np.concatenate([q_b[sl], kv_b[sl], kv_b[vsl], qb_mu, kb_mu])[None, :]
        in_maps.append({
            "hT": hT[b],
            "qwT": np.ascontiguousarray(q_w[sl].T).astype(bf16),
            "kwT": np.ascontiguousarray(kv_w[sl].T).astype(bf16),
            "vwT": np.ascontiguousarray(kv_w[vsl].T).astype(bf16),
            "owT": np.ascontiguousarray(o_w[sl].T).astype(bf16),
            "muwT": np.ascontiguousarray(muw.T).astype(bf16),
            "qkvb": np.ascontiguousarray(qkvb_row).astype(bf16),
            "ob": np.ascontiguousarray(np.broadcast_to(o_b[sl], (P, OC))),
            "cosd": cos,
            "sind": sin,
        })
    return in_maps


def _assemble(results):
    out = np.empty((B, S, H), np.float32)
    for c in range(NCORES):
        b, hg = divmod(c, 4)
        out[b, :, hg * OC : (hg + 1) * OC] = results[c]["out"]
    return out


def _enable_ldw_opt():
    try:
        from concourse.compiler_utils import get_compiler_flags, set_compiler_flags
        flags = get_compiler_flags()
        patched = [f.replace("--enable-ldw-opt=false", "--enable-ldw-opt=true") for f in flags]
        if patched != flags:
            set_compiler_flags(patched)
    except Exception:
        pass


def kernel(**inputs):
    from concourse.bass_utils import run_bass_kernel_spmd

    _enable_ldw_opt()

    nc = _get_nc()
    in_maps = _prep_in_maps(inputs)
    res = run_bass_kernel_spmd(nc, in_maps, list(range(NCORES)))
    results = res.results if hasattr(res, "results") else res
    return _assemble(results)



# revision 8
# speedup vs baseline: 1.1013x; 1.1013x over previous
"""Trainium2 Bass kernel for fused attention block (B=2, S=2048, H=1024, N=16, D=64).

Sharding: 8 cores = 2 batches (DP) x 4 head-groups (TP, 4 heads each).

v2 design vs the previous baseline:
- LN mean-subtract folded into host-centered projection weights (exact), so
  the mu ride-along columns, mean broadcasts and subtracts all disappear.
- Attention restructured as (s-quarter, head-pair) sweeps: QK uses K=64
  stationaries in the two row halves of the PE array (tile_position packing,
  both heads' scores stream concurrently), scores psum is [128,512] so the
  whole phase fits in 7 psum banks with double buffering.
- Normalized outputs ship per (s-quarter, head-pair) through 8 small
  AllGathers that overlap attention; the output projection is emitted per
  quarter one sweep behind, so only the last quarter's tail is exposed.
- exp is split ACT/DVE(Schraudolph)+gpsimd-cast with a tunable share.
- All sqrt calls happen during the projection phase, so ACT switches
  activation tables exactly once (sqrt set -> exp set).
"""

import numpy as np
import ml_dtypes
from contextlib import ExitStack

import concourse.bass as bass
from concourse import bacc
import concourse.mybir as mybir
import concourse.tile as tile
from concourse.masks import make_identity

# problem shape (hardcoded per contract)
B, S, H, NH, D = 2, 2048, 1024, 16, 64
EPS = 1.0 / 65530.0
NCORES = 8
HPC = 4            # heads per core
OC = HPC * D       # 256 head-dims per core
P = 128
SB = S // P        # 16 s-blocks
KC = H // P        # 8 contraction chunks of 128
D2 = D // 2
SCALE = 1.0 / 8.0  # 1/sqrt(D)
DV = D + 1         # V columns per head incl. ones column
SQ = 512           # s-quarter width
NSQ = S // SQ      # 4
GS = 4             # s-blocks per chain group (= per quarter)
PW = 3 * OC        # projection psum width (q|k|v)

BF = mybir.dt.bfloat16
F32 = mybir.dt.float32
I32 = mybir.dt.int32
ALU = mybir.AluOpType
ACTF = mybir.ActivationFunctionType

# Schraudolph fast-exp constants (int32 bit trick)
EXP_A = float(2**23 / np.log(2))
EXP_B = float(127 * 2**23 - 366400)
# of the 16 per-t exp ops per sweep, how many go to the DVE (rest on ACT)
DVE_SHARE = {0: 5, 1: 6, 2: 6, 3: 6}


def _dve_slot(idx, share):
    # Bresenham spread of `share` DVE slots over 16
    return ((idx + 1) * share) // 16 > (idx * share) // 16


def build_nc():
    nc = bacc.Bacc(num_devices=NCORES)

    hT = nc.declare_dram_parameter("hT", [H, S], BF, isOutput=False)
    qkvwT = nc.declare_dram_parameter("qkvwT", [H, PW], BF, isOutput=False)
    owT = nc.declare_dram_parameter("owT", [H, OC], BF, isOutput=False)
    qkvb = nc.declare_dram_parameter("qkvb", [1, PW], BF, isOutput=False)
    obr = nc.declare_dram_parameter("obr", [1, OC], BF, isOutput=False)
    cosd = nc.declare_dram_parameter("cosd", [P, SB * D], BF, isOutput=False)
    sind = nc.declare_dram_parameter("sind", [P, SB * D], BF, isOutput=False)
    out = nc.declare_dram_parameter("out", [S, OC], F32, isOutput=True)

    with tile.TileContext(nc) as tc:
        with tc.tile_pool(name="persist", bufs=1) as persist, \
             tc.tile_pool(name="dram", bufs=1, space="DRAM") as dram:
            # warm-up fodder + identity first so PE can start immediately
            junk = persist.tile([P, 512], BF)
            nc.gpsimd.memset(junk[:], 1.0)
            ident = persist.tile([P, P], BF)
            make_identity(nc, ident)
            onesrow = persist.tile([1, P], BF)
            nc.gpsimd.memset(onesrow[:], 1.0)
            eps_t = persist.tile([P, 1], F32)
            nc.gpsimd.memset(eps_t[:], EPS)

            # input DMAs: weights first (gate the first proj matmul), then hT
            # in s-chunks; owT afterwards (needed only in phase O)
            qkvwT_sb = persist.tile([P, KC, PW], BF)
            hT_sb = persist.tile([P, KC, S], BF)
            HTC = 4
            SCH = S // HTC
            nc.sync.dma_start(qkvwT_sb[:, 0], qkvwT[0:P].rearrange("(a p) o -> p (a o)", a=1))
            nc.sync.dma_start(
                hT_sb[:, :, 0:SCH],
                hT[:, 0:SCH].rearrange("(a p) s -> p a s", p=P),
            )
            for kc in range(1, KC):
                nc.sync.dma_start(
                    qkvwT_sb[:, kc], qkvwT[kc * P : (kc + 1) * P].rearrange("(a p) o -> p (a o)", a=1)
                )
            for hc in range(1, HTC):
                nc.sync.dma_start(
                    hT_sb[:, :, hc * SCH : (hc + 1) * SCH],
                    hT[:, hc * SCH : (hc + 1) * SCH].rearrange("(a p) s -> p a s", p=P),
                )
            owT_sb = persist.tile([P, KC, OC], BF)
            nc.sync.dma_start(owT_sb[:], owT[:].rearrange("(a p) o -> p a o", p=P))

            cos_sb = persist.tile([P, SB, D], BF)
            nc.scalar.dma_start(cos_sb[:], cosd[:].rearrange("p (a d) -> p a d", d=D))
            sin_sb = persist.tile([P, SB, D], BF)
            nc.scalar.dma_start(sin_sb[:], sind[:].rearrange("p (a d) -> p a d", d=D))
            qkvb_sb = persist.tile([1, PW], BF)
            nc.scalar.dma_start(qkvb_sb[:], qkvb[:])
            obr_sb = persist.tile([1, OC], BF)
            nc.scalar.dma_start(obr_sb[:], obr[:])

            # cos/sin pre-broadcast over heads (bf16, contiguous for 2x DVE)
            cos_f = persist.tile([P, SB, HPC, D], BF)
            nc.vector.tensor_copy(
                out=cos_f[:], in_=cos_sb[:, :, None, :].to_broadcast((P, SB, HPC, D))
            )
            sin_f = persist.tile([P, SB, HPC, D], BF)
            nc.vector.tensor_copy(
                out=sin_f[:], in_=sin_sb[:, :, None, :].to_broadcast((P, SB, HPC, D))
            )

            # persistent activations
            qkf = persist.tile([P, SB, 2 * OC], BF)   # q | k projections
            Vp = persist.tile([P, SB, HPC * DV], BF)  # v + ones col per head
            for h in range(HPC):
                nc.gpsimd.memset(Vp[:, :, h * DV + D : (h + 1) * DV], 1.0)
            qT2 = persist.tile([P, 2, S], BF)  # pair c: rows 0-63 head 2c, 64-127 head 2c+1
            kT2 = persist.tile([P, 2, S], BF)
            attnT = persist.tile([D, HPC, S], BF)  # normalized attn [d, h, s]

            var_q = persist.tile([P, SB, HPC], F32)
            var_k = persist.tile([P, SB, HPC], F32)
            std_q = persist.tile([P, SB, HPC], F32)
            std_k = persist.tile([P, SB, HPC], F32)
            rstd_q = persist.tile([P, SB, HPC], F32)
            rstd_k = persist.tile([P, SB, HPC], F32)
            rstd_qb = persist.tile([P, SB, HPC], BF)
            rstd_kb = persist.tile([P, SB, HPC], BF)

            # collective bounce buffers: one per (s-quarter, head-pair)
            cc_in = [[dram.tile([P, SQ], BF, name=f"ccin{sq}{c}") for c in range(2)]
                     for sq in range(NSQ)]
            cc_out = [[dram.tile([4 * P, SQ], BF, name=f"ccout{sq}{c}") for c in range(2)]
                      for sq in range(NSQ)]

            with tc.tile_pool(name="ctmp", bufs=2) as ctmp:

                def stats_grp(xoff, g, var, std, rstd, rstd_b, tagp, scale):
                    gs = slice(g * GS, (g + 1) * GS)
                    xf = qkf[:, gs, xoff : xoff + OC]
                    sqf = ctmp.tile([P, GS, OC], F32, name=f"sqf{tagp}{g}", tag="sqf", bufs=2)
                    nc.gpsimd.tensor_tensor(out=sqf[:], in0=xf, in1=xf, op=ALU.mult)
                    sv = sqf[:].rearrange("p s (h d) -> p s h d", h=HPC)
                    nc.vector.tensor_reduce(out=var[:, gs], in_=sv, axis=mybir.AxisListType.X, op=ALU.add)
                    nc.vector.tensor_scalar_mul(var[:, gs], var[:, gs], 1.0 / D)
                    nc.scalar.activation(std[:, gs], var[:, gs], ACTF.Sqrt, bias=eps_t[:])
                    nc.vector.reciprocal(rstd[:, gs], std[:, gs])
                    if scale != 1.0:
                        nc.vector.tensor_scalar_mul(rstd[:, gs], rstd[:, gs], scale)
                    nc.vector.tensor_copy(out=rstd_b[:, gs], in_=rstd[:, gs])

                def rope_grp(xoff, g, rstd_b, tagp):
                    # returns rx2: rope(x) * rstd, laid out [p, si, (h d)]
                    gs = slice(g * GS, (g + 1) * GS)
                    xv = qkf[:, gs, xoff : xoff + OC].rearrange("p s (h d) -> p s h d", h=HPC)
                    cb = cos_f[:, gs]
                    s1 = sin_f[:, gs, :, 0:D2]
                    s2 = sin_f[:, gs, :, D2:D]
                    ca = ctmp.tile([P, GS, HPC, D], BF, name=f"ca{tagp}{g}", tag="ca", bufs=2)
                    th = ctmp.tile([P, GS, HPC, D2], BF, name=f"th{tagp}{g}", tag="th", bufs=2)
                    t2 = ctmp.tile([P, GS, HPC, D2], BF, name=f"t2{tagp}{g}", tag="t2", bufs=2)
                    rx = ctmp.tile([P, GS, HPC, D], BF, name=f"rx{tagp}{g}", tag="rx", bufs=2)
                    nc.gpsimd.tensor_tensor(out=th[:], in0=xv[:, :, :, D2:D], in1=s1, op=ALU.mult)
                    nc.gpsimd.tensor_tensor(out=t2[:], in0=xv[:, :, :, 0:D2], in1=s2, op=ALU.mult)
                    nc.gpsimd.tensor_tensor(out=ca[:], in0=xv, in1=cb, op=ALU.mult)
                    nc.vector.tensor_tensor(out=rx[:, :, :, 0:D2], in0=ca[:, :, :, 0:D2], in1=th[:], op=ALU.subtract)
                    nc.vector.tensor_tensor(out=rx[:, :, :, D2:D], in0=ca[:, :, :, D2:D], in1=t2[:], op=ALU.add)
                    # scale by rstd: expand to a contiguous bf16 tile, then TT
                    rs_f = ctmp.tile([P, GS, HPC, D], BF, name=f"rsf{tagp}{g}", tag="rsf", bufs=2)
                    nc.vector.tensor_copy(
                        out=rs_f[:], in_=rstd_b[:, gs, :, None].to_broadcast((P, GS, HPC, D))
                    )
                    nc.vector.tensor_tensor(out=rx[:], in0=rx[:], in1=rs_f[:], op=ALU.mult)
                    return rx[:].rearrange("p s h d -> p s (h d)")

                def k_apply(g):
                    rx2 = rope_grp(OC, g, rstd_kb, "k")
                    for c in range(2):
                        for si in range(GS):
                            sb = g * GS + si
                            nc.sync.dma_start(
                                kT2[:, c, sb * P : (sb + 1) * P],
                                rx2[:, si, c * P : (c + 1) * P],
                                transpose=True,
                            )

                # ---------------- phase Q: projection ----------------
                with tc.tile_pool(name="projpsum", bufs=3, space="PSUM") as projpsum:
                    # PE warm-up burst bridges the initial DMA wait
                    wps = projpsum.tile([P, PW], F32, name="wps", tag="pq")
                    for _ in range(12):
                        nc.tensor.matmul(wps[:, 0:512], ident[:], junk[:], start=True, stop=True)

                    def proj_sb(sb):
                        pq = projpsum.tile([P, PW], F32, name=f"pq{sb}", tag="pq")
                        for kc in range(KC):
                            lhsp = hT_sb[:, kc, sb * P : (sb + 1) * P]
                            nc.tensor.matmul(
                                pq[:, 0:512], lhsp, qkvwT_sb[:, kc, 0:512],
                                start=(kc == 0), stop=False,
                            )
                            nc.tensor.matmul(
                                pq[:, 512:PW], lhsp, qkvwT_sb[:, kc, 512:PW],
                                start=(kc == 0), stop=False,
                            )
                        nc.tensor.matmul(
                            pq[:, 0:512], onesrow[:], qkvb_sb[:, 0:512],
                            start=False, stop=True,
                        )
                        nc.tensor.matmul(
                            pq[:, 512:PW], onesrow[:], qkvb_sb[:, 512:PW],
                            start=False, stop=True,
                        )
                        nc.scalar.copy(out=qkf[:, sb], in_=pq[:, 0:512])
                        nc.vector.tensor_copy(
                            out=Vp[:, sb].rearrange("p (h e) -> p h e", h=HPC)[:, :, 0:D],
                            in_=pq[:, 512:PW].rearrange("p (h d) -> p h d", h=HPC),
                        )

                    for g in range(4):
                        for si in range(GS):
                            proj_sb(g * GS + si)
                        stats_grp(OC, g, var_k, std_k, rstd_k, rstd_kb, "k", 1.0)
                        k_apply(g)
                        stats_grp(0, g, var_q, std_q, rstd_q, rstd_qb, "q", SCALE)

                    # prefetch the exp activation table (all sqrts are done)
                    dummy = ctmp.tile([1, 2], F32, name="dummy", tag="dummy", bufs=1)
                    nc.scalar.activation(dummy[:], junk[0:1, 0:2], ACTF.Exp)

                # ---------------- phase A + O ----------------
                def q_apply(tpsum, g):
                    rx2 = rope_grp(0, g, rstd_qb, "q")
                    for c in range(2):
                        for si in range(GS):
                            sb = g * GS + si
                            pst = tpsum.tile([P, P], BF, name=f"pst{c}{sb}", tag="pst")
                            nc.tensor.transpose(pst[:], rx2[:, si, c * P : (c + 1) * P], ident[:])
                            nc.vector.tensor_copy(out=qT2[:, c, sb * P : (sb + 1) * P], in_=pst[:])

                def qk(sc_ap, sq, hp, t, half):
                    lhs = kT2[half * 64 : (half + 1) * 64, hp, t * P : (t + 1) * P]
                    rhs = qT2[half * 64 : (half + 1) * 64, hp, sq * SQ : (sq + 1) * SQ]
                    nc.tensor.matmul(sc_ap, lhs, rhs, start=True, stop=True)

                def exp_emit(atmp, probs_t, sc_t, sq, hp, t, dve):
                    # one op covers both heads' scores [128, 2*SQ]
                    if dve:
                        it = atmp.tile([P, 2 * SQ], I32, name=f"it{sq}{hp}{t}", tag="it", bufs=2)
                        nc.vector.tensor_scalar(
                            out=it[:], in0=sc_t[:], scalar1=EXP_A, scalar2=EXP_B,
                            op0=ALU.mult, op1=ALU.add,
                        )
                        nc.vector.tensor_copy(out=probs_t[:], in_=it[:].bitcast(F32))
                    else:
                        nc.scalar.activation(probs_t[:], sc_t[:], ACTF.Exp)

                def pv(h, t, pvp, probs_ap):
                    nc.tensor.matmul(
                        pvp[:], Vp[:, t, h * DV : (h + 1) * DV], probs_ap,
                        start=(t == 0), stop=(t == SB - 1),
                    )

                def normalize(atmp, pvp, c, half, sq):
                    h = 2 * c + half
                    pvf = atmp.tile([DV, SQ], F32, name=f"pvf{sq}{h}", tag="pvf", bufs=2)
                    nc.scalar.copy(out=pvf[:], in_=pvp[:])
                    rb = atmp.tile([D, SQ], F32, name=f"rb{sq}{h}", tag="rb", bufs=2)
                    # two-stage tree broadcast of the sums row (1->8, 8->64)
                    nc.sync.dma_start(rb[0:8, :], pvf[D : D + 1, None, :].to_broadcast((1, 8, SQ)))
                    for i in range(7):
                        nc.sync.dma_start(rb[8 * (i + 1) : 8 * (i + 2), :], rb[0:8, :])
                    nc.vector.reciprocal_approx_fast(rb[:], rb[:])
                    nc.vector.tensor_tensor(
                        out=attnT[:, h, sq * SQ : (sq + 1) * SQ],
                        in0=pvf[0:D, :], in1=rb[:], op=ALU.mult,
                    )

                def ship(sq, c):
                    nc.gpsimd.dma_start(
                        cc_in[sq][c][:].rearrange("(hh p) s -> p hh s", p=D),
                        attnT[:, 2 * c : 2 * c + 2, sq * SQ : (sq + 1) * SQ],
                    )
                    nc.gpsimd.collective_compute(
                        "AllGather", ALU.bypass,
                        replica_groups=[[0, 1, 2, 3], [4, 5, 6, 7]],
                        ins=[cc_in[sq][c][:].opt()], outs=[cc_out[sq][c][:].opt()],
                    )

                def sweep(spsum, pvpsum, probspool, atmp, sq, hp):
                    share = DVE_SHARE[sq]
                    hA, hB = 2 * hp, 2 * hp + 1
                    pvpA = pvpsum.tile([DV, SQ], F32, name=f"pvA{sq}{hp}", tag="pvp")
                    pvpB = pvpsum.tile([DV, SQ], F32, name=f"pvB{sq}{hp}", tag="pvp")
                    probs = {}
                    for t in range(SB):
                        sc = spsum.tile([P, 2 * SQ], F32, name=f"sc{sq}{hp}{t}", tag="sc")
                        qk(sc[:, 0:SQ], sq, hp, t, 0)
                        qk(sc[:, SQ : 2 * SQ], sq, hp, t, 1)
                        pr = probspool.tile([P, 2 * SQ], BF, name=f"pr{sq}{hp}{t}", tag="pr")
                        exp_emit(atmp, pr, sc, sq, hp, t, _dve_slot(t, share))
                        probs[t] = pr
                        if t >= 1:
                            prv = probs.pop(t - 1)
                            pv(hA, t - 1, pvpA, prv[:, 0:SQ])
                            pv(hB, t - 1, pvpB, prv[:, SQ : 2 * SQ])
                    prv = probs.pop(SB - 1)
                    pv(hA, SB - 1, pvpA, prv[:, 0:SQ])
                    pv(hB, SB - 1, pvpB, prv[:, SQ : 2 * SQ])
                    normalize(atmp, pvpA, hp, 0, sq)
                    normalize(atmp, pvpB, hp, 1, sq)
                    ship(sq, hp)

                def oproj(opsum, otmp, atmp, sq):
                    aTc = []
                    for c in range(2):
                        a = atmp.tile([P, 4, SQ], BF, name=f"aT{sq}{c}", tag="aT", bufs=2)
                        nc.gpsimd.dma_start(
                            a[:], cc_out[sq][c][:].rearrange("(g p) s -> p g s", p=P)
                        )
                        aTc.append(a)
                    for sbl in range(4):
                        sb = sq * 4 + sbl
                        pso = opsum.tile([P, OC], F32, name=f"pso{sb}", tag="pso")
                        first = True
                        for g in range(4):
                            for c in range(2):
                                nc.tensor.matmul(
                                    pso[:],
                                    aTc[c][:, g, sbl * P : (sbl + 1) * P],
                                    owT_sb[:, 2 * g + c],
                                    start=first, stop=False,
                                )
                                first = False
                        nc.tensor.matmul(
                            pso[:], onesrow[:], obr_sb[:], start=False, stop=True,
                        )
                        of = otmp.tile([P, OC], F32, name=f"of{sb}", tag="of", bufs=2)
                        nc.scalar.copy(out=of[:], in_=pso[:])
                        nc.sync.dma_start(out[sb * P : (sb + 1) * P, :], of[:])

                # sq0/sq1 under tpsum (q transposes); sq2/sq3 under opsum
                with tc.tile_pool(name="tpsum", bufs=2, space="PSUM") as tpsum:
                    q_apply(tpsum, 0)
                    with tc.tile_pool(name="spsum", bufs=2, space="PSUM") as spsum, \
                         tc.tile_pool(name="pvpsum", bufs=2, space="PSUM") as pvpsum, \
                         tc.tile_pool(name="probs", bufs=4) as probspool, \
                         tc.tile_pool(name="atmp", bufs=2) as atmp:
                        sweep(spsum, pvpsum, probspool, atmp, 0, 0)
                        q_apply(tpsum, 1)
                        sweep(spsum, pvpsum, probspool, atmp, 0, 1)
                        q_apply(tpsum, 2)
                        sweep(spsum, pvpsum, probspool, atmp, 1, 0)
                        q_apply(tpsum, 3)
                        sweep(spsum, pvpsum, probspool, atmp, 1, 1)
                with tc.tile_pool(name="opsum", bufs=2, space="PSUM") as opsum:
                    with tc.tile_pool(name="spsum2", bufs=2, space="PSUM") as spsum2, \
                         tc.tile_pool(name="pvpsum2", bufs=2, space="PSUM") as pvpsum2, \
                         tc.tile_pool(name="probs2", bufs=4) as probspool2, \
                         tc.tile_pool(name="atmp2", bufs=2) as atmp2, \
                         tc.tile_pool(name="otmp", bufs=2) as otmp:
                        oproj(opsum, otmp, atmp2, 0)
                        sweep(spsum2, pvpsum2, probspool2, atmp2, 2, 0)
                        sweep(spsum2, pvpsum2, probspool2, atmp2, 2, 1)
                        oproj(opsum, otmp, atmp2, 1)
                        sweep(spsum2, pvpsum2, probspool2, atmp2, 3, 0)
                        sweep(spsum2, pvpsum2, probspool2, atmp2, 3, 1)
                        oproj(opsum, otmp, atmp2, 2)
                        oproj(opsum, otmp, atmp2, 3)

    nc.finalize()
    return nc


_NC_CACHE = None


def _get_nc():
    global _NC_CACHE
    if _NC_CACHE is None:
        _NC_CACHE = build_nc()
    return _NC_CACHE


def _prep_in_maps(inputs):
    bf16 = ml_dtypes.bfloat16
    hidden = np.asarray(inputs["hidden_states"], np.float32)
    cos = np.ascontiguousarray(np.asarray(inputs["cos"], np.float32))
    sin = np.ascontiguousarray(np.asarray(inputs["sin"], np.float32))
    q_w = np.asarray(inputs["q_w"], np.float32)
    q_b = np.asarray(inputs["q_b"], np.float32)
    kv_w = np.asarray(inputs["kv_w"], np.float32)
    kv_b = np.asarray(inputs["kv_b"], np.float32)
    o_w = np.asarray(inputs["o_w"], np.float32)
    o_b = np.asarray(inputs["o_b"], np.float32)

    hT = [np.ascontiguousarray(hidden[b].T).astype(bf16) for b in range(B)]
    # prepack cos/sin to [P, SB*D]: partition p holds s-blocks' rows for s = a*P + p
    cos_pk = np.ascontiguousarray(
        cos.reshape(SB, P, D).transpose(1, 0, 2).reshape(P, SB * D)
    ).astype(bf16)
    sin_pk = np.ascontiguousarray(
        sin.reshape(SB, P, D).transpose(1, 0, 2).reshape(P, SB * D)
    ).astype(bf16)

    in_maps = []
    for c in range(NCORES):
        b, hg = divmod(c, 4)
        sl = slice(hg * OC, (hg + 1) * OC)
        vsl = slice(H + hg * OC, H + (hg + 1) * OC)
        # center q/k weight rows per head: LN mean-subtract folded into W
        qw = q_w[sl].reshape(HPC, D, H)
        qw = (qw - qw.mean(axis=1, keepdims=True)).reshape(OC, H)
        kw = kv_w[sl].reshape(HPC, D, H)
        kw = (kw - kw.mean(axis=1, keepdims=True)).reshape(OC, H)
        vw = kv_w[vsl]
        qb = q_b[sl].reshape(HPC, D)
        qb = (qb - qb.mean(axis=1, keepdims=True)).reshape(OC)
        kb = kv_b[sl].reshape(HPC, D)
        kb = (kb - kb.mean(axis=1, keepdims=True)).reshape(OC)
        qkvw = np.concatenate([qw, kw, vw], axis=0)          # [768, H]
        qkvb_row = np.concatenate([qb, kb, kv_b[vsl]])[None, :]
        in_maps.append({
            "hT": hT[b],
            "qkvwT": np.ascontiguousarray(qkvw.T).astype(bf16),
            "owT": np.ascontiguousarray(o_w[sl].T).astype(bf16),
            "qkvb": np.ascontiguousarray(qkvb_row).astype(bf16),
            "obr": np.ascontiguousarray(o_b[sl][None, :]).astype(bf16),
            "cosd": cos_pk,
            "sind": sin_pk,
        })
    return in_maps


def _assemble(results):
    out = np.empty((B, S, H), np.float32)
    for c in range(NCORES):
        b, hg = divmod(c, 4)
        out[b, :, hg * OC : (hg + 1) * OC] = results[c]["out"]
    return out


def _enable_ldw_opt():
    try:
        from concourse.compiler_utils import get_compiler_flags, set_compiler_flags
        flags = get_compiler_flags()
        patched = [f.replace("--enable-ldw-opt=false", "--enable-ldw-opt=true") for f in flags]
        if patched != flags:
            set_compiler_flags(patched)
    except Exception:
        pass


def kernel(**inputs):
    from concourse.bass_utils import run_bass_kernel_spmd

    _enable_ldw_opt()

    nc = _get_nc()
    in_maps = _prep_in_maps(inputs)
    res = run_bass_kernel_spmd(nc, in_maps, list(range(NCORES)))
    results = res.results if hasattr(res, "results") else res
    return _assemble(results)


# revision 16
# speedup vs baseline: 1.1335x; 1.0292x over previous
"""Trainium2 Bass kernel for fused attention block (B=2, S=2048, H=1024, N=16, D=64).

Sharding: 8 cores = 2 batches (DP) x 4 head-groups (TP, 4 heads each).

v2 design vs the previous baseline:
- LN mean-subtract folded into host-centered projection weights (exact), so
  the mu ride-along columns, mean broadcasts and subtracts all disappear.
- Attention restructured as (s-quarter, head-pair) sweeps: QK uses K=64
  stationaries in the two row halves of the PE array (tile_position packing,
  both heads' scores stream concurrently), scores psum is [128,512] so the
  whole phase fits in 7 psum banks with double buffering.
- Normalized outputs ship per (s-quarter, head-pair) through 8 small
  AllGathers that overlap attention; the output projection is emitted per
  quarter one sweep behind, so only the last quarter's tail is exposed.
- exp is split ACT/DVE(Schraudolph)+gpsimd-cast with a tunable share.
- All sqrt calls happen during the projection phase, so ACT switches
  activation tables exactly once (sqrt set -> exp set).
"""

import numpy as np
import ml_dtypes
from contextlib import ExitStack

import concourse.bass as bass
from concourse import bacc
import concourse.mybir as mybir
import concourse.tile as tile
from concourse.masks import make_identity

# problem shape (hardcoded per contract)
B, S, H, NH, D = 2, 2048, 1024, 16, 64
EPS = 1.0 / 65530.0
NCORES = 8
HPC = 4            # heads per core
OC = HPC * D       # 256 head-dims per core
P = 128
SB = S // P        # 16 s-blocks
KC = H // P        # 8 contraction chunks of 128
D2 = D // 2
SCALE = 1.0 / 8.0  # 1/sqrt(D)
DV = D + 1         # V columns per head incl. ones column
SQ = 512           # s-quarter width
NSQ = S // SQ      # 4
GS = 4             # s-blocks per chain group (= per quarter)
PW = 3 * OC        # projection psum width (q|k|v)

BF = mybir.dt.bfloat16
F32 = mybir.dt.float32
I32 = mybir.dt.int32
ALU = mybir.AluOpType
ACTF = mybir.ActivationFunctionType

# Schraudolph fast-exp constants (int32 bit trick)
EXP_A = float(2**23 / np.log(2))
EXP_B = float(127 * 2**23 - 366400)
# of the 16 per-t exp ops per sweep, how many go to the DVE (rest on ACT)
DVE_SHARE = {0: 5, 1: 6, 2: 6, 3: 6}


def _dve_slot(idx, share):
    # Bresenham spread of `share` DVE slots over 16
    return ((idx + 1) * share) // 16 > (idx * share) // 16


def build_nc():
    nc = bacc.Bacc(num_devices=NCORES)

    hT = nc.declare_dram_parameter("hT", [H, S], BF, isOutput=False)
    qkvwT = nc.declare_dram_parameter("qkvwT", [H, PW], BF, isOutput=False)
    owT = nc.declare_dram_parameter("owT", [H, OC], BF, isOutput=False)
    qkvb = nc.declare_dram_parameter("qkvb", [1, PW], BF, isOutput=False)
    obr = nc.declare_dram_parameter("obr", [1, OC], BF, isOutput=False)
    cosd = nc.declare_dram_parameter("cosd", [P, SB * D], BF, isOutput=False)
    sind = nc.declare_dram_parameter("sind", [P, SB * D], BF, isOutput=False)
    out = nc.declare_dram_parameter("out", [S, OC], F32, isOutput=True)

    with tile.TileContext(nc) as tc:
        with tc.tile_pool(name="persist", bufs=1) as persist, \
             tc.tile_pool(name="dram", bufs=1, space="DRAM") as dram:
            # warm-up fodder + identity first so PE can start immediately
            junk = persist.tile([P, 512], BF)
            nc.gpsimd.memset(junk[:], 1.0)
            ident = persist.tile([P, P], BF)
            make_identity(nc, ident)
            onesrow = persist.tile([1, P], BF)
            nc.gpsimd.memset(onesrow[:], 1.0)
            eps_t = persist.tile([P, 1], F32)
            nc.gpsimd.memset(eps_t[:], EPS)

            # input DMAs: weights first (gate the first proj matmul), then hT
            # in s-chunks; owT afterwards (needed only in phase O)
            qkvwT_sb = persist.tile([P, KC, PW], BF)
            hT_sb = persist.tile([P, KC, S], BF)
            HTC = 4
            SCH = S // HTC
            nc.sync.dma_start(qkvwT_sb[:, 0], qkvwT[0:P].rearrange("(a p) o -> p (a o)", a=1))
            nc.sync.dma_start(
                hT_sb[:, :, 0:SCH],
                hT[:, 0:SCH].rearrange("(a p) s -> p a s", p=P),
            )
            for kc in range(1, KC):
                nc.sync.dma_start(
                    qkvwT_sb[:, kc], qkvwT[kc * P : (kc + 1) * P].rearrange("(a p) o -> p (a o)", a=1)
                )
            for hc in range(1, HTC):
                nc.sync.dma_start(
                    hT_sb[:, :, hc * SCH : (hc + 1) * SCH],
                    hT[:, hc * SCH : (hc + 1) * SCH].rearrange("(a p) s -> p a s", p=P),
                )
            owT_sb = persist.tile([P, KC, OC], BF)
            nc.sync.dma_start(owT_sb[:], owT[:].rearrange("(a p) o -> p a o", p=P))

            cos_sb = persist.tile([P, SB, D], BF)
            nc.scalar.dma_start(cos_sb[:], cosd[:].rearrange("p (a d) -> p a d", d=D))
            sin_sb = persist.tile([P, SB, D], BF)
            nc.scalar.dma_start(sin_sb[:], sind[:].rearrange("p (a d) -> p a d", d=D))
            qkvb_sb = persist.tile([1, PW], BF)
            nc.scalar.dma_start(qkvb_sb[:], qkvb[:])
            obr_sb = persist.tile([1, OC], BF)
            nc.scalar.dma_start(obr_sb[:], obr[:])

            # cos/sin pre-broadcast over heads (bf16, contiguous for 2x DVE)
            cos_f = persist.tile([P, SB, HPC, D], BF)
            nc.vector.tensor_copy(
                out=cos_f[:], in_=cos_sb[:, :, None, :].to_broadcast((P, SB, HPC, D))
            )
            sin_f = persist.tile([P, SB, HPC, D], BF)
            nc.vector.tensor_copy(
                out=sin_f[:], in_=sin_sb[:, :, None, :].to_broadcast((P, SB, HPC, D))
            )

            # persistent activations
            qkf = persist.tile([P, SB, 2 * OC], BF)   # q | k projections
            Vp = persist.tile([P, SB, HPC * DV], BF)  # v + ones col per head
            for h in range(HPC):
                nc.gpsimd.memset(Vp[:, :, h * DV + D : (h + 1) * DV], 1.0)
            qT2 = persist.tile([P, 2, S], BF)  # pair c: rows 0-63 head 2c, 64-127 head 2c+1
            kT2 = persist.tile([P, 2, S], BF)
            attnT = persist.tile([D, HPC, S], BF)  # normalized attn [d, h, s]

            var_q = persist.tile([P, SB, HPC], F32)
            var_k = persist.tile([P, SB, HPC], F32)
            std_q = persist.tile([P, SB, HPC], F32)
            std_k = persist.tile([P, SB, HPC], F32)
            rstd_q = persist.tile([P, SB, HPC], F32)
            rstd_k = persist.tile([P, SB, HPC], F32)
            rstd_qb = persist.tile([P, SB, HPC], BF)
            rstd_kb = persist.tile([P, SB, HPC], BF)

            # collective bounce buffers: one per s-quarter (all 4 local heads)
            cc_in = [dram.tile([2 * P, SQ], BF, name=f"ccin{sq}") for sq in range(3)]
            cc_out = [dram.tile([8 * P, SQ], BF, name=f"ccout{sq}") for sq in range(3)]
            cc_in3 = [dram.tile([P, SQ], BF, name=f"ccin3{c}") for c in range(2)]
            cc_out3 = [dram.tile([4 * P, SQ], BF, name=f"ccout3{c}") for c in range(2)]

            with tc.tile_pool(name="ctmp", bufs=2) as ctmp:

                def stats_grp(xoff, g, var, std, rstd, rstd_b, tagp, scale):
                    gs = slice(g * GS, (g + 1) * GS)
                    xf = qkf[:, gs, xoff : xoff + OC]
                    sqf = ctmp.tile([P, GS, OC], F32, name=f"sqf{tagp}{g}", tag="sqf", bufs=2)
                    nc.gpsimd.tensor_tensor(out=sqf[:], in0=xf, in1=xf, op=ALU.mult)
                    sv = sqf[:].rearrange("p s (h d) -> p s h d", h=HPC)
                    nc.vector.tensor_reduce(out=var[:, gs], in_=sv, axis=mybir.AxisListType.X, op=ALU.add)
                    nc.vector.tensor_scalar_mul(var[:, gs], var[:, gs], 1.0 / D)
                    nc.scalar.activation(std[:, gs], var[:, gs], ACTF.Sqrt, bias=eps_t[:])
                    nc.vector.reciprocal(rstd[:, gs], std[:, gs])
                    if scale != 1.0:
                        nc.vector.tensor_scalar_mul(rstd[:, gs], rstd[:, gs], scale)
                    nc.vector.tensor_copy(out=rstd_b[:, gs], in_=rstd[:, gs])

                def rope_grp(xoff, g, rstd_b, tagp):
                    # returns rx2: rope(x) * rstd, laid out [p, si, (h d)]
                    gs = slice(g * GS, (g + 1) * GS)
                    xv = qkf[:, gs, xoff : xoff + OC].rearrange("p s (h d) -> p s h d", h=HPC)
                    cb = cos_f[:, gs]
                    s1 = sin_f[:, gs, :, 0:D2]
                    s2 = sin_f[:, gs, :, D2:D]
                    ca = ctmp.tile([P, GS, HPC, D], BF, name=f"ca{tagp}{g}", tag="ca", bufs=2)
                    th = ctmp.tile([P, GS, HPC, D2], BF, name=f"th{tagp}{g}", tag="th", bufs=2)
                    t2 = ctmp.tile([P, GS, HPC, D2], BF, name=f"t2{tagp}{g}", tag="t2", bufs=2)
                    rx = ctmp.tile([P, GS, HPC, D], BF, name=f"rx{tagp}{g}", tag="rx", bufs=2)
                    nc.gpsimd.tensor_tensor(out=th[:], in0=xv[:, :, :, D2:D], in1=s1, op=ALU.mult)
                    nc.gpsimd.tensor_tensor(out=t2[:], in0=xv[:, :, :, 0:D2], in1=s2, op=ALU.mult)
                    nc.vector.tensor_tensor(out=ca[:], in0=xv, in1=cb, op=ALU.mult)
                    nc.vector.tensor_tensor(out=rx[:, :, :, 0:D2], in0=ca[:, :, :, 0:D2], in1=th[:], op=ALU.subtract)
                    nc.vector.tensor_tensor(out=rx[:, :, :, D2:D], in0=ca[:, :, :, D2:D], in1=t2[:], op=ALU.add)
                    # scale by rstd: expand to a contiguous bf16 tile, then TT
                    rs_f = ctmp.tile([P, GS, HPC, D], BF, name=f"rsf{tagp}{g}", tag="rsf", bufs=2)
                    nc.vector.tensor_copy(
                        out=rs_f[:], in_=rstd_b[:, gs, :, None].to_broadcast((P, GS, HPC, D))
                    )
                    nc.vector.tensor_tensor(out=rx[:], in0=rx[:], in1=rs_f[:], op=ALU.mult)
                    return rx[:].rearrange("p s h d -> p s (h d)")

                def k_apply(g):
                    rx2 = rope_grp(OC, g, rstd_kb, "k")
                    for c in range(2):
                        for si in range(GS):
                            sb = g * GS + si
                            eng = nc.sync if (si % 2 == 0) else nc.scalar
                            eng.dma_start(
                                kT2[:, c, sb * P : (sb + 1) * P],
                                rx2[:, si, c * P : (c + 1) * P],
                                transpose=True,
                            )

                def q_apply(tpsum, g):
                    rx2 = rope_grp(0, g, rstd_qb, "q")
                    for c in range(2):
                        for si in range(GS):
                            sb = g * GS + si
                            pst = tpsum.tile([P, P], BF, name=f"pst{c}{sb}", tag="pst")
                            nc.tensor.transpose(pst[:], rx2[:, si, c * P : (c + 1) * P], ident[:])
                            nc.vector.tensor_copy(out=qT2[:, c, sb * P : (sb + 1) * P], in_=pst[:])

                # ---------------- phase Q: projection ----------------
                tpsum_cm = tc.tile_pool(name="tpsum", bufs=2, space="PSUM")
                tpsum = tpsum_cm.__enter__()
                with tc.tile_pool(name="projpsum", bufs=3, space="PSUM") as projpsum:
                    # PE warm-up burst bridges the initial DMA wait
                    wps = projpsum.tile([P, PW], F32, name="wps", tag="pq")
                    for _ in range(12):
                        nc.tensor.matmul(wps[:, 0:512], ident[:], junk[:], start=True, stop=True)

                    def proj_sb(sb):
                        pq = projpsum.tile([P, PW], F32, name=f"pq{sb}", tag="pq")
                        for kc in range(KC):
                            lhsp = hT_sb[:, kc, sb * P : (sb + 1) * P]
                            nc.tensor.matmul(
                                pq[:, 0:512], lhsp, qkvwT_sb[:, kc, 0:512],
                                start=(kc == 0), stop=False,
                            )
                            nc.tensor.matmul(
                                pq[:, 512:PW], lhsp, qkvwT_sb[:, kc, 512:PW],
                                start=(kc == 0), stop=False,
                            )
                        nc.tensor.matmul(
                            pq[:, 0:512], onesrow[:], qkvb_sb[:, 0:512],
                            start=False, stop=True,
                        )
                        nc.tensor.matmul(
                            pq[:, 512:PW], onesrow[:], qkvb_sb[:, 512:PW],
                            start=False, stop=True,
                        )
                        nc.scalar.copy(out=qkf[:, sb], in_=pq[:, 0:512])
                        nc.vector.tensor_copy(
                            out=Vp[:, sb].rearrange("p (h e) -> p h e", h=HPC)[:, :, 0:D],
                            in_=pq[:, 512:PW].rearrange("p (h d) -> p h d", h=HPC),
                        )

                    for g in range(4):
                        for si in range(GS):
                            proj_sb(g * GS + si)
                        stats_grp(OC, g, var_k, std_k, rstd_k, rstd_kb, "k", 1.0)
                        k_apply(g)
                        stats_grp(0, g, var_q, std_q, rstd_q, rstd_qb, "q", SCALE)
                        if g == 0:
                            q_apply(tpsum, 0)

                    # prefetch the exp activation table (all sqrts are done)
                    dummy = ctmp.tile([1, 2], F32, name="dummy", tag="dummy", bufs=1)
                    nc.scalar.activation(dummy[:], junk[0:1, 0:2], ACTF.Exp)

                # ---------------- phase A + O ----------------
                def qk(sc_ap, sq, hp, t, half):
                    lhs = kT2[half * 64 : (half + 1) * 64, hp, t * P : (t + 1) * P]
                    rhs = qT2[half * 64 : (half + 1) * 64, hp, sq * SQ : (sq + 1) * SQ]
                    nc.tensor.matmul(sc_ap, lhs, rhs, start=True, stop=True)

                def exp_emit(atmp, probs_t, sc_t, sq, hp, t, dve):
                    # one op covers both heads' scores [128, 2*SQ]
                    if dve:
                        it = atmp.tile([P, 2 * SQ], I32, name=f"it{sq}{hp}{t}", tag="it", bufs=2)
                        nc.vector.tensor_scalar(
                            out=it[:], in0=sc_t[:], scalar1=EXP_A, scalar2=EXP_B,
                            op0=ALU.mult, op1=ALU.add,
                        )
                        nc.vector.tensor_copy(out=probs_t[:], in_=it[:].bitcast(F32))
                    else:
                        nc.scalar.activation(probs_t[:], sc_t[:], ACTF.Exp)

                def pv(h, t, pvp, probs_ap):
                    nc.tensor.matmul(
                        pvp[:], Vp[:, t, h * DV : (h + 1) * DV], probs_ap,
                        start=(t == 0), stop=(t == SB - 1),
                    )

                pending_norm = []

                def flush_norm():
                    for fn in pending_norm:
                        fn()
                    pending_norm.clear()

                def normalize(atmp, pvp, c, half, sq):
                    # start now: evacuate psum + reciprocal of the sums row;
                    # defer the broadcast matmul + multiply so they hide
                    # behind the next sweep's first QK blocks
                    h = 2 * c + half
                    pvf = atmp.tile([DV, SQ], F32, name=f"pvf{sq}{h}", tag="pvf", bufs=2)
                    nc.scalar.copy(out=pvf[:], in_=pvp[:])
                    rb = atmp.tile([D, SQ], F32, name=f"rb{sq}{h}", tag="rb", bufs=2)
                    nc.sync.dma_start(rb[0:8, :], pvf[D : D + 1, None, :].to_broadcast((1, 8, SQ)))
                    for i in range(7):
                        nc.sync.dma_start(rb[8 * (i + 1) : 8 * (i + 2), :], rb[0:8, :])
                    nc.vector.reciprocal_approx_fast(rb[:], rb[:])
                    nc.vector.tensor_tensor(
                        out=attnT[:, h, sq * SQ : (sq + 1) * SQ],
                        in0=pvf[0:D, :], in1=rb[:], op=ALU.mult,
                    )

                def ship(sq):
                    nc.gpsimd.dma_start(
                        cc_in[sq][:].rearrange("(hh p) s -> p hh s", p=D),
                        attnT[:, :, sq * SQ : (sq + 1) * SQ],
                    )
                    nc.gpsimd.collective_compute(
                        "AllGather", ALU.bypass,
                        replica_groups=[[0, 1, 2, 3], [4, 5, 6, 7]],
                        ins=[cc_in[sq][:].opt()], outs=[cc_out[sq][:].opt()],
                    )

                def ship3(c):
                    # last quarter ships per head-pair so only the second
                    # pair's 128KB gather is exposed after attention ends
                    nc.gpsimd.dma_start(
                        cc_in3[c][:].rearrange("(hh p) s -> p hh s", p=D),
                        attnT[:, 2 * c : 2 * c + 2, 3 * SQ : 4 * SQ],
                    )
                    nc.gpsimd.collective_compute(
                        "AllGather", ALU.bypass,
                        replica_groups=[[0, 1, 2, 3], [4, 5, 6, 7]],
                        ins=[cc_in3[c][:].opt()], outs=[cc_out3[c][:].opt()],
                    )

                def sweep(spsum, pvpsum, probspool, atmp, sq, hp):
                    share = DVE_SHARE[sq]
                    hA, hB = 2 * hp, 2 * hp + 1
                    pvpA = pvpsum.tile([DV, SQ], F32, name=f"pvA{sq}{hp}", tag="pvp")
                    pvpB = pvpsum.tile([DV, SQ], F32, name=f"pvB{sq}{hp}", tag="pvp")
                    probs = {}
                    for t in range(SB):
                        sc = spsum.tile([P, 2 * SQ], F32, name=f"sc{sq}{hp}{t}", tag="sc")
                        qk(sc[:, 0:SQ], sq, hp, t, 0)
                        qk(sc[:, SQ : 2 * SQ], sq, hp, t, 1)
                        pr = probspool.tile([P, 2 * SQ], BF, name=f"pr{sq}{hp}{t}", tag="pr")
                        exp_emit(atmp, pr, sc, sq, hp, t, _dve_slot(t, share))
                        probs[t] = pr
                        if t == 2:
                            flush_norm()
                        if t >= 2:
                            prv = probs.pop(t - 2)
                            pv(hA, t - 2, pvpA, prv[:, 0:SQ])
                            pv(hB, t - 2, pvpB, prv[:, SQ : 2 * SQ])
                    for tt in (SB - 2, SB - 1):
                        prv = probs.pop(tt)
                        pv(hA, tt, pvpA, prv[:, 0:SQ])
                        pv(hB, tt, pvpB, prv[:, SQ : 2 * SQ])
                    normalize(atmp, pvpA, hp, 0, sq)
                    normalize(atmp, pvpB, hp, 1, sq)
                    if sq == 3:
                        ship3(hp)
                    elif hp == 1:
                        ship(sq)

                def oproj(opsum, otmp, atmp, sq):
                    if sq == 3:
                        a3 = []
                        for c in range(2):
                            ac = atmp.tile([P, 4, SQ], BF, name=f"aT3{c}", tag="aT3", bufs=2)
                            nc.sync.dma_start(
                                ac[:], cc_out3[c][:].rearrange("(g p) s -> p g s", p=P)
                            )
                            a3.append(ac)
                        chunk = lambda g, c, sbl: a3[c][:, g, sbl * P : (sbl + 1) * P]
                    else:
                        a = atmp.tile([P, 8, SQ], BF, name=f"aT{sq}", tag="aT", bufs=2)
                        nc.sync.dma_start(
                            a[:], cc_out[sq][:].rearrange("(j p) s -> p j s", p=P)
                        )
                        chunk = lambda g, c, sbl: a[:, 2 * g + c, sbl * P : (sbl + 1) * P]
                    for sbl in range(4):
                        sb = sq * 4 + sbl
                        pso = opsum.tile([P, OC], F32, name=f"pso{sb}", tag="pso")
                        first = True
                        for g in range(4):
                            for c in range(2):
                                nc.tensor.matmul(
                                    pso[:],
                                    chunk(g, c, sbl),
                                    owT_sb[:, 2 * g + c],
                                    start=first, stop=False,
                                )
                                first = False
                        nc.tensor.matmul(
                            pso[:], onesrow[:], obr_sb[:], start=False, stop=True,
                        )
                        of = otmp.tile([P, OC], F32, name=f"of{sb}", tag="of", bufs=2)
                        nc.scalar.copy(out=of[:], in_=pso[:])
                        nc.sync.dma_start(out[sb * P : (sb + 1) * P, :], of[:])

                # sq0/sq1 under tpsum (q transposes); sq2/sq3 under opsum
                if True:
                    with tc.tile_pool(name="spsum", bufs=2, space="PSUM") as spsum, \
                         tc.tile_pool(name="pvpsum", bufs=2, space="PSUM") as pvpsum, \
                         tc.tile_pool(name="probs", bufs=4) as probspool, \
                         tc.tile_pool(name="atmp", bufs=2) as atmp:
                        sweep(spsum, pvpsum, probspool, atmp, 0, 0)
                        q_apply(tpsum, 1)
                        sweep(spsum, pvpsum, probspool, atmp, 0, 1)
                        q_apply(tpsum, 2)
                        sweep(spsum, pvpsum, probspool, atmp, 1, 0)
                        q_apply(tpsum, 3)
                        sweep(spsum, pvpsum, probspool, atmp, 1, 1)
                    tpsum_cm.__exit__(None, None, None)
                with tc.tile_pool(name="opsum", bufs=2, space="PSUM") as opsum:
                    with tc.tile_pool(name="spsum2", bufs=2, space="PSUM") as spsum2, \
                         tc.tile_pool(name="pvpsum2", bufs=2, space="PSUM") as pvpsum2, \
                         tc.tile_pool(name="probs2", bufs=4) as probspool2, \
                         tc.tile_pool(name="atmp2", bufs=2) as atmp2, \
                         tc.tile_pool(name="otmp", bufs=2) as otmp:
                        oproj(opsum, otmp, atmp2, 0)
                        sweep(spsum2, pvpsum2, probspool2, atmp2, 2, 0)
                        sweep(spsum2, pvpsum2, probspool2, atmp2, 2, 1)
                        oproj(opsum, otmp, atmp2, 1)
                        sweep(spsum2, pvpsum2, probspool2, atmp2, 3, 0)
                        sweep(spsum2, pvpsum2, probspool2, atmp2, 3, 1)
                        oproj(opsum, otmp, atmp2, 2)
                        oproj(opsum, otmp, atmp2, 3)

    nc.finalize()
    return nc


_NC_CACHE = None


def _get_nc():
    global _NC_CACHE
    if _NC_CACHE is None:
        _NC_CACHE = build_nc()
    return _NC_CACHE


def _prep_in_maps(inputs):
    bf16 = ml_dtypes.bfloat16
    hidden = np.asarray(inputs["hidden_states"], np.float32)
    cos = np.ascontiguousarray(np.asarray(inputs["cos"], np.float32))
    sin = np.ascontiguousarray(np.asarray(inputs["sin"], np.float32))
    q_w = np.asarray(inputs["q_w"], np.float32)
    q_b = np.asarray(inputs["q_b"], np.float32)
    kv_w = np.asarray(inputs["kv_w"], np.float32)
    kv_b = np.asarray(inputs["kv_b"], np.float32)
    o_w = np.asarray(inputs["o_w"], np.float32)
    o_b = np.asarray(inputs["o_b"], np.float32)

    hT = [np.ascontiguousarray(hidden[b].T).astype(bf16) for b in range(B)]
    # prepack cos/sin to [P, SB*D]: partition p holds s-blocks' rows for s = a*P + p
    cos_pk = np.ascontiguousarray(
        cos.reshape(SB, P, D).transpose(1, 0, 2).reshape(P, SB * D)
    ).astype(bf16)
    sin_pk = np.ascontiguousarray(
        sin.reshape(SB, P, D).transpose(1, 0, 2).reshape(P, SB * D)
    ).astype(bf16)

    in_maps = []
    for c in range(NCORES):
        b, hg = divmod(c, 4)
        sl = slice(hg * OC, (hg + 1) * OC)
        vsl = slice(H + hg * OC, H + (hg + 1) * OC)
        # center q/k weight rows per head: LN mean-subtract folded into W
        qw = q_w[sl].reshape(HPC, D, H)
        qw = (qw - qw.mean(axis=1, keepdims=True)).reshape(OC, H)
        kw = kv_w[sl].reshape(HPC, D, H)
        kw = (kw - kw.mean(axis=1, keepdims=True)).reshape(OC, H)
        vw = kv_w[vsl]
        qb = q_b[sl].reshape(HPC, D)
        qb = (qb - qb.mean(axis=1, keepdims=True)).reshape(OC)
        kb = kv_b[sl].reshape(HPC, D)
        kb = (kb - kb.mean(axis=1, keepdims=True)).reshape(OC)
        qkvw = np.concatenate([qw, kw, vw], axis=0)          # [768, H]
        qkvb_row = np.concatenate([qb, kb, kv_b[vsl]])[None, :]
        in_maps.append({
            "hT": hT[b],
            "qkvwT": np.ascontiguousarray(qkvw.T).astype(bf16),
            "owT": np.ascontiguousarray(o_w[sl].T).astype(bf16),
            "qkvb": np.ascontiguousarray(qkvb_row).astype(bf16),
            "obr": np.ascontiguousarray(o_b[sl][None, :]).astype(bf16),
            "cosd": cos_pk,
            "sind": sin_pk,
        })
    return in_maps


def _assemble(results):
    out = np.empty((B, S, H), np.float32)
    for c in range(NCORES):
        b, hg = divmod(c, 4)
        out[b, :, hg * OC : (hg + 1) * OC] = results[c]["out"]
    return out


def _enable_ldw_opt():
    try:
        from concourse.compiler_utils import get_compiler_flags, set_compiler_flags
        flags = get_compiler_flags()
        patched = [f.replace("--enable-ldw-opt=false", "--enable-ldw-opt=true") for f in flags]
        if patched != flags:
            set_compiler_flags(patched)
    except Exception:
        pass


def kernel(**inputs):
    from concourse.bass_utils import run_bass_kernel_spmd

    _enable_ldw_opt()

    nc = _get_nc()
    in_maps = _prep_in_maps(inputs)
    res = run_bass_kernel_spmd(nc, in_maps, list(range(NCORES)))
    results = res.results if hasattr(res, "results") else res
    return _assemble(results)
